# revision 22
# baseline (speedup 1.0000x reference)
"""Cross-attention Trainium2 kernel, tensor-parallel over 8 NeuronCores.

Sharding: core c handles batch b = c // 4 and head-group hg = c % 4
(4 heads = 512 of the 2048 hidden dims). Each core computes its heads'
QKV projections, RoPE, softmax attention (transposed-scores layout), and
a partial output projection. The host sums the 4 partials per batch.

Schedule (PE-roofline oriented):
  P0  all Q projections + Q RoPE, overlapped with every input DMA
  P1  V projection (PSUM drains on ACT, which is otherwise idle)
  P2  per head h: attention over the 4 q-chunks with the NEXT head's
      K-projection matmuls interleaved into the exp-paced gaps; softmax
      denominator via DVE pair sums + lagged ones-matmuls on PE;
      normalize (GPSIMD broadcast + reciprocal_approx_fast + mul)
      deferred one pass so its latency never stalls PE
  P3  output projection, drains split ACT/DVE, bf16 partial DMA

Self-contained: hardcodes all shapes from the problem spec.
"""

import numpy as np
import ml_dtypes

import concourse.bacc as bacc
import concourse.bass_isa as bass_isa
import concourse.tile as tile
from concourse import mybir
from concourse.bass_utils import run_bass_kernel_spmd

BF16 = ml_dtypes.bfloat16

B, T, S = 2, 2048, 2048
QD, CD = 2048, 2048
H, D = 16, 128
NE = 64            # rotary dims
KVMAX = 2048
N_CORES = 8
N_HG = 4           # head groups (cores per batch)
HPC = H // N_HG    # heads per core = 4
HD = HPC * D       # 512 head dims per core
KT = CD // 128     # contraction tiles = 16
ST = KVMAX // 128  # kv tiles = 16
CW = 512           # q chunk width
NCH = T // CW      # 4 chunks
SCALE = float(D) ** -0.5
IDENT32 = list(range(32))

DT_B = mybir.dt.bfloat16
DT_F = mybir.dt.float32

_compiled = {}


def _build(reps=1):
    if reps in _compiled:
        return _compiled[reps]

    nc = bacc.Bacc("TRN2", target_bir_lowering=False, debug=False,
                   num_devices=N_CORES)

    xT = nc.dram_tensor("xT", [128, NCH, KT, CW], DT_B, kind="ExternalInput")
    yT = nc.dram_tensor("yT", [128, KT, KVMAX], DT_B, kind="ExternalInput")
    wqT = nc.dram_tensor("wqT", [128, KT, HD], DT_B, kind="ExternalInput")
    wkT = nc.dram_tensor("wkT", [128, KT, HD], DT_B, kind="ExternalInput")
    wvT = nc.dram_tensor("wvT", [128, KT, HD], DT_B, kind="ExternalInput")
    woT = nc.dram_tensor("woT", [128, HPC, QD], DT_B, kind="ExternalInput")
    cosq = nc.dram_tensor("cosq", [NE, T], DT_B, kind="ExternalInput")
    sinq = nc.dram_tensor("sinq", [NE, T], DT_B, kind="ExternalInput")
    cosk = nc.dram_tensor("cosk", [NE, KVMAX], DT_B, kind="ExternalInput")
    sink = nc.dram_tensor("sink", [NE, KVMAX], DT_B, kind="ExternalInput")
    partial = nc.dram_tensor("partial", [T, QD], DT_B, kind="ExternalOutput")

    with tile.TileContext(nc) as tc:
        if reps == 1:
            _body(nc, tc, xT, yT, wqT, wkT, wvT, woT, cosq, sinq, cosk,
                  sink, partial)
        else:
            with tc.For_i(0, reps, 1):
                _body(nc, tc, xT, yT, wqT, wkT, wvT, woT, cosq, sinq,
                      cosk, sink, partial)

    nc.compile()
    _compiled[reps] = nc
    return nc


def _rope(nc, pool, dst, cos_sb, sin_sb, w):
    """In-place RoPE on dst[0:NE, :w] (head-dim on partitions).

    cos_sb/sin_sb are [NE, w] slices; sin rows 0:32 carry -sin, 32:64 +sin.
    """
    rot = pool.tile([NE, w], DT_B, tag="rot")
    half = NE // 2
    nc.vector.stream_shuffle(rot[0:half, :], dst[half:NE, :], IDENT32)
    nc.vector.stream_shuffle(rot[half:NE, :], dst[0:half, :], IDENT32)
    nc.vector.tensor_mul(rot[:, :], rot[:, :], sin_sb)
    nc.vector.tensor_mul(dst[0:NE, :], dst[0:NE, :], cos_sb)
    nc.vector.tensor_add(dst[0:NE, :], dst[0:NE, :], rot[:, :])


def _body(nc, tc, xT, yT, wqT, wkT, wvT, woT, cosq, sinq, cosk, sink,
          partial):
    from contextlib import ExitStack

    with ExitStack() as ctx:
        const = ctx.enter_context(tc.tile_pool(name="const", bufs=1))
        qpool = ctx.enter_context(tc.tile_pool(name="q", bufs=1))
        kvpool = ctx.enter_context(tc.tile_pool(name="kv", bufs=1))
        ktab = ctx.enter_context(tc.tile_pool(name="ktab", bufs=1))
        rope_pool = ctx.enter_context(tc.tile_pool(name="rope", bufs=2))
        # PSUM: pps 2 + sps 1x4 + avps 2 = 8 banks
        pps = ctx.enter_context(tc.tile_pool(name="pps", bufs=2,
                                             space="PSUM"))
        sps = ctx.enter_context(tc.tile_pool(name="sps", bufs=1,
                                             space="PSUM"))
        avps = ctx.enter_context(tc.tile_pool(name="avps", bufs=2,
                                              space="PSUM"))

        zbias = const.tile([128, 1], DT_F)
        nc.gpsimd.memset(zbias[:], 0.0)
        ones_col = const.tile([128, 1], DT_B)
        nc.gpsimd.memset(ones_col[:], 1.0)

        q_all = qpool.tile([128, HPC, T], DT_B)
        k_sb = kvpool.tile([128, HPC, KVMAX], DT_B)
        v_sb = kvpool.tile([128, ST, HD], DT_B)
        cosk_sb = ktab.tile([NE, KVMAX], DT_B)
        sink_sb = ktab.tile([NE, KVMAX], DT_B)
        nc.sync.dma_start(cosk_sb[:], cosk.ap())
        nc.sync.dma_start(sink_sb[:], sink.ap())

        # ---- P0: all Q projections (+ RoPE) while x streams ----
        with ExitStack() as c0:
            wqpool = c0.enter_context(tc.tile_pool(name="wq", bufs=1))
            xpool = c0.enter_context(tc.tile_pool(name="x", bufs=2))
            qtab = c0.enter_context(tc.tile_pool(name="qtab", bufs=1))

            wq_sb = wqpool.tile([128, KT, HD], DT_B)
            x_tiles = []
            for _ in range(NCH):
                x_sb = xpool.tile([128, KT, CW], DT_B, tag="x")
                x_tiles.append(x_sb)
            # interleave wq/x0 pieces k-tile-major: the first qproj matmuls
            # (kk=0..3) can start once the first two pieces land
            for g in range(4):
                nc.sync.dma_start(wq_sb[:, g * 4:(g + 1) * 4, :],
                                  wqT.ap()[:, g * 4:(g + 1) * 4, :])
                nc.sync.dma_start(x_tiles[0][:, g * 4:(g + 1) * 4, :],
                                  xT.ap()[:, 0, g * 4:(g + 1) * 4, :])
            cosq_sb = qtab.tile([NE, T], DT_B)
            nc.sync.dma_start(cosq_sb[:], cosq.ap())
            sinq_sb = qtab.tile([NE, T], DT_B)
            nc.sync.dma_start(sinq_sb[:], sinq.ap())
            for c in range(1, NCH):
                for g in range(4):
                    nc.sync.dma_start(x_tiles[c][:, g * 4:(g + 1) * 4, :],
                                      xT.ap()[:, c, g * 4:(g + 1) * 4, :])

            for ci in range(NCH):
                x_sb = x_tiles[ci]
                for h in range(HPC):
                    qp = pps.tile([128, CW], DT_F, tag="pp")
                    for kk in range(KT):
                        nc.tensor.matmul(
                            qp[:], wq_sb[:, kk, h * 128:(h + 1) * 128],
                            x_sb[:, kk, :],
                            start=(kk == 0), stop=(kk == KT - 1))
                    dst = q_all[:, h, ci * CW:(ci + 1) * CW]
                    if h % 2 == 0:
                        nc.vector.tensor_copy(dst, qp[:])
                    else:
                        nc.scalar.copy(dst, qp[:])
                    _rope(nc, rope_pool, dst,
                          cosq_sb[:, ci * CW:(ci + 1) * CW],
                          sinq_sb[:, ci * CW:(ci + 1) * CW], CW)

        # y / wk / wv / wo loads issued behind the P0 x DMAs
        ypool = ctx.enter_context(tc.tile_pool(name="y", bufs=1))
        wkpool = ctx.enter_context(tc.tile_pool(name="wk", bufs=1))
        y_sb = ypool.tile([128, KT, KVMAX], DT_B)
        for g in range(8):
            nc.sync.dma_start(y_sb[:, g * 2:(g + 1) * 2, :],
                              yT.ap()[:, g * 2:(g + 1) * 2, :])
        wk_sb = wkpool.tile([128, KT, HD], DT_B)
        for g in range(4):
            nc.sync.dma_start(wk_sb[:, g * 4:(g + 1) * 4, :],
                              wkT.ap()[:, g * 4:(g + 1) * 4, :])

        with ExitStack() as c1:
            wvpool = c1.enter_context(tc.tile_pool(name="wv", bufs=1))
            wv_sb = wvpool.tile([128, KT, HD], DT_B)
            for g in range(4):
                nc.sync.dma_start(wv_sb[:, g * 4:(g + 1) * 4, :],
                                  wvT.ap()[:, g * 4:(g + 1) * 4, :])

            # ---- P1: V projection; drains on ACT (idle here) ----
            for st in range(ST):
                vp = pps.tile([128, HD], DT_F, tag="pp")
                for kk in range(KT):
                    nc.tensor.matmul(
                        vp[:], y_sb[:, kk, st * 128:(st + 1) * 128],
                        wv_sb[:, kk, :],
                        start=(kk == 0), stop=(kk == KT - 1))
                nc.scalar.copy(v_sb[:, st, :], vp[:])

        # ---- P2: per-head attention with next head's K proj woven in ----
        o_tiles = {}
        opool = ctx.enter_context(tc.tile_pool(name="o", bufs=16))
        wopool = ctx.enter_context(tc.tile_pool(name="wo", bufs=1))
        wo_sb = wopool.tile([128, HPC, QD], DT_B)
        for g in range(HPC):
            nc.sync.dma_start(wo_sb[:, g, :], woT.ap()[:, g, :])
        with ExitStack() as c2:
            apool = c2.enter_context(tc.tile_pool(name="attn", bufs=2))
            lpool = c2.enter_context(tc.tile_pool(name="leaf", bufs=4))
            dpool = c2.enter_context(tc.tile_pool(name="den", bufs=1))
            dsbp = c2.enter_context(tc.tile_pool(name="dsb", bufs=2))

            def kproj_group(h, sc, kk_lo, kk_hi, kp):
                """Emit K-proj matmuls kk_lo..kk_hi for (head h, chunk sc)."""
                for kk in range(kk_lo, kk_hi):
                    nc.tensor.matmul(
                        kp[:], wk_sb[:, kk, h * 128:(h + 1) * 128],
                        y_sb[:, kk, sc * CW:(sc + 1) * CW],
                        start=(kk == 0), stop=(kk == KT - 1))

            def kproj_finish(h, sc, kp):
                dst = k_sb[:, h, sc * CW:(sc + 1) * CW]
                nc.vector.tensor_copy(dst, kp[:])
                _rope(nc, rope_pool, dst,
                      cosk_sb[:, sc * CW:(sc + 1) * CW],
                      sink_sb[:, sc * CW:(sc + 1) * CW], CW)

            # K proj head 0 upfront (drains on DVE; ACT idle)
            for sc in range(NCH):
                kp = pps.tile([128, CW], DT_F, tag="pp")
                kproj_group(0, sc, 0, KT, kp)
                kproj_finish(0, sc, kp)

            # normalize for (c, h) emitted one attention pass later: the
            # GPSIMD tree result p1 isn't ready when its own pass ends, so
            # the denominator matmul would stall PE if issued inline
            pending = []

            def emit_normalize():
                c, h, den_sb, av = pending.pop(0)
                den_bc = dpool.tile([128, CW], DT_F, tag="dbc")
                nc.gpsimd.partition_broadcast(den_bc[:], den_sb[:])
                nc.vector.reciprocal_approx_fast(den_bc[:], den_bc[:])
                o_sb = opool.tile([128, CW], DT_B, tag="o")
                nc.vector.tensor_mul(o_sb[:], den_bc[:], av[:])
                o_tiles[(c, h)] = o_sb

            for h in range(HPC):
                for c in range(NCH):
                    # interleaved K proj (h+1, chunk c): 16 MMs woven into
                    # this attention pass, 2 per exp-paced gap
                    ikp = None
                    if h + 1 < HPC:
                        ikp = pps.tile([128, CW], DT_F, tag="pp")

                    q_ap = q_all[:, h, c * CW:(c + 1) * CW]
                    av = avps.tile([128, CW], DT_F, tag="av")
                    den = pps.tile([1, CW], DT_F, tag="pp")
                    leaves = []
                    at_prev = None
                    for sg in range(ST // 4):
                        sp = sps.tile([128, 4, CW], DT_F, tag="sp")
                        for j in range(4):
                            st = 4 * sg + j
                            nc.tensor.matmul(
                                sp[:, j, :],
                                k_sb[:, h, st * 128:(st + 1) * 128],
                                q_ap, start=True, stop=True)
                        if ikp is not None:
                            kproj_group(h + 1, c, 4 * sg, 4 * sg + 4, ikp)
                        if at_prev is not None:
                            for j in range(4):
                                st = 4 * (sg - 1) + j
                                nc.tensor.matmul(
                                    av[:],
                                    v_sb[:, st, h * 128:(h + 1) * 128],
                                    at_prev[:, j, :], start=(st == 0),
                                    stop=False)
                        # den matmuls lag their leaf by one super-group
                        if len(leaves) >= 2:
                            lp = leaves[len(leaves) - 2]
                            for j in range(2):
                                nc.tensor.matmul(
                                    den[:], ones_col[:], lp[:, j, :],
                                    start=(len(leaves) == 2 and j == 0),
                                    stop=False)
                        if sg == 2 and pending:
                            emit_normalize()
                        at = apool.tile([128, 4, CW], DT_B, tag="at")
                        nc.scalar.activation(
                            at[:, :, :], sp[:, :, :],
                            mybir.ActivationFunctionType.Exp, bias=zbias[:])
                        ps = lpool.tile([128, 2, CW], DT_B, tag="ps")
                        nc.vector.tensor_add(ps[:, :, :], at[:, 0:2, :],
                                             at[:, 2:4, :])
                        leaves.append(ps)
                        at_prev = at
                    # tail: av for the last super-group, then the lagged den
                    # MMs — all before kproj_finish's DVE burst
                    for j in range(4):
                        st = ST - 4 + j
                        nc.tensor.matmul(
                            av[:], v_sb[:, st, h * 128:(h + 1) * 128],
                            at_prev[:, j, :], start=False,
                            stop=(st == ST - 1))
                    for i in (2, 3):
                        for j in range(2):
                            nc.tensor.matmul(
                                den[:], ones_col[:], leaves[i][:, j, :],
                                start=False, stop=(i == 3 and j == 1))
                    den_sb = dsbp.tile([1, CW], DT_F, tag="dsb")
                    nc.scalar.copy(den_sb[:], den[:])
                    if ikp is not None:
                        kproj_finish(h + 1, c, ikp)
                    pending.append((c, h, den_sb, av))

            while pending:
                emit_normalize()

        # ---- P3: output projection; fp tiles rotate over pps+avps
        # (4 single-bank buffers) so drains never gate the matmul stream ----
        with tc.tile_pool(name="part", bufs=3) as ppart:
            for c in range(NCH):
                for qt in range(CW // 128):
                    part_sb = ppart.tile([128, QD], DT_B, tag="part")
                    for nt in range(QD // 512):
                        pool = pps if nt % 2 == 0 else avps
                        tag = "pp" if nt % 2 == 0 else "av"
                        fp = pool.tile([128, 512], DT_F, tag=tag)
                        for h in range(HPC):
                            nc.tensor.matmul(
                                fp[:],
                                o_tiles[(c, h)][:, qt * 128:(qt + 1) * 128],
                                wo_sb[:, h, nt * 512:(nt + 1) * 512],
                                start=(h == 0), stop=(h == HPC - 1))
                        dst = part_sb[:, nt * 512:(nt + 1) * 512]
                        if nt % 2 == 0:
                            nc.vector.tensor_copy(dst, fp[:])
                        else:
                            nc.scalar.copy(dst, fp[:])
                    row0 = c * CW + qt * 128
                    nc.sync.dma_start(partial[row0:row0 + 128, :],
                                      part_sb[:])


def _tile_rows(a, p=128):
    """[R, M] with R = n*p  ->  [p, n, M] (partition-major tiling)."""
    r, m = a.shape
    return np.ascontiguousarray(
        a.reshape(r // p, p, m).transpose(1, 0, 2))


def _host_shards(inputs):
    """Build the 8 per-core input maps from the full inputs."""
    x = np.asarray(inputs["x"], np.float32)
    y = np.asarray(inputs["y"], np.float32)
    rope_cos = np.asarray(inputs["rope_cos"], np.float32)
    rope_sin = np.asarray(inputs["rope_sin"], np.float32)
    wq = np.asarray(inputs["wq"], np.float32)
    wk = np.asarray(inputs["wk"], np.float32)
    wv = np.asarray(inputs["wv"], np.float32)
    wo = np.asarray(inputs["wo"], np.float32)
    input_pos = np.asarray(inputs["input_pos"], np.int64)

    # KV-cache scatter folded into a host-side permutation of y's rows and
    # of the rope tables (k positions live at cache slot input_pos[s]).
    y_cache = np.zeros((B, KVMAX, CD), np.float32)
    y_cache[:, input_pos, :] = y
    ck = np.zeros((KVMAX, NE // 2), np.float32)
    ck[input_pos] = rope_cos
    sk = np.zeros((KVMAX, NE // 2), np.float32)
    sk[input_pos] = rope_sin

    def tabT(cos2, sin2):
        cosT = np.tile(cos2.T, (2, 1)).astype(BF16)          # [NE, S]
        sinT = np.concatenate([-sin2.T, sin2.T], 0).astype(BF16)
        return np.ascontiguousarray(cosT), np.ascontiguousarray(sinT)

    cosq_h, sinq_h = tabT(rope_cos[:T], rope_sin[:T])
    cosk_h, sink_h = tabT(ck, sk)

    in_maps = []
    for core in range(N_CORES):
        b, hg = core // N_HG, core % N_HG
        rows = slice(hg * HD, (hg + 1) * HD)
        xt = _tile_rows(x[b].T.astype(BF16))        # [128, KT, T]
        xt = np.ascontiguousarray(
            xt.reshape(128, KT, NCH, CW).transpose(0, 2, 1, 3))
        in_maps.append({
            "xT": xt,                               # [128, NCH, KT, CW]
            "yT": _tile_rows(y_cache[b].T.astype(BF16)),
            "wqT": _tile_rows((wq[rows] * SCALE).T.astype(BF16)),
            "wkT": _tile_rows(wk[rows].T.astype(BF16)),
            "wvT": _tile_rows(wv[rows].T.astype(BF16)),
            "woT": _tile_rows(wo[:, rows].T.astype(BF16)),
            "cosq": cosq_h, "sinq": sinq_h,
            "cosk": cosk_h, "sink": sink_h,
        })
    return in_maps


def _run(inputs, trace=False, reps=1, **kw):
    nc = _build(reps)
    in_maps = _host_shards(inputs)
    res = run_bass_kernel_spmd(nc, in_maps, list(range(N_CORES)),
                               trace=trace, **kw)
    out = np.zeros((B, T, QD), np.float32)
    for core in range(N_CORES):
        out[core // N_HG] += np.asarray(res.results[core]["partial"],
                                        dtype=np.float32)
    return out, res


def kernel(**inputs):
    out, _ = _run(inputs)
    return out


# revision 26
# speedup vs baseline: 1.0181x; 1.0181x over previous
"""Cross-attention Trainium2 kernel, tensor-parallel over 8 NeuronCores.

Sharding: core c handles batch b = c // 4 and head-group hg = c % 4
(4 heads = 512 of the 2048 hidden dims). Each core computes its heads'
QKV projections, RoPE, softmax attention (transposed-scores layout), and
a partial output projection. The host sums the 4 partials per batch.

Schedule (PE-roofline oriented):
  P0  all Q projections + Q RoPE, overlapped with every input DMA
  P1  V projection (PSUM drains on ACT, which is otherwise idle)
  P2  per head h: attention over the 4 q-chunks with the NEXT head's
      K-projection matmuls interleaved into the exp-paced gaps; softmax
      denominator via DVE pair sums + lagged ones-matmuls on PE;
      normalize (GPSIMD broadcast + reciprocal_approx_fast + mul)
      deferred one pass so its latency never stalls PE
  P3  output projection, drains split ACT/DVE, bf16 partial DMA

Self-contained: hardcodes all shapes from the problem spec.
"""

import numpy as np
import ml_dtypes

import concourse.bacc as bacc
import concourse.bass_isa as bass_isa
import concourse.tile as tile
from concourse import mybir
from concourse.bass_utils import run_bass_kernel_spmd

BF16 = ml_dtypes.bfloat16

B, T, S = 2, 2048, 2048
QD, CD = 2048, 2048
H, D = 16, 128
NE = 64            # rotary dims
KVMAX = 2048
N_CORES = 8
N_HG = 4           # head groups (cores per batch)
HPC = H // N_HG    # heads per core = 4
HD = HPC * D       # 512 head dims per core
KT = CD // 128     # contraction tiles = 16
ST = KVMAX // 128  # kv tiles = 16
CW = 512           # q chunk width
NCH = T // CW      # 4 chunks
SCALE = float(D) ** -0.5
IDENT32 = list(range(32))

DT_B = mybir.dt.bfloat16
DT_F = mybir.dt.float32

_compiled = {}


def _build(reps=1):
    if reps in _compiled:
        return _compiled[reps]

    nc = bacc.Bacc("TRN2", target_bir_lowering=False, debug=False,
                   num_devices=N_CORES)

    xT = nc.dram_tensor("xT", [128, NCH, KT, CW], DT_B, kind="ExternalInput")
    yT = nc.dram_tensor("yT", [128, KT, KVMAX], DT_B, kind="ExternalInput")
    wqT = nc.dram_tensor("wqT", [128, KT, HD], DT_B, kind="ExternalInput")
    wkT = nc.dram_tensor("wkT", [128, KT, HD], DT_B, kind="ExternalInput")
    wvT = nc.dram_tensor("wvT", [128, KT, HD], DT_B, kind="ExternalInput")
    woT = nc.dram_tensor("woT", [128, HPC, QD], DT_B, kind="ExternalInput")
    cosq = nc.dram_tensor("cosq", [NE, T], DT_B, kind="ExternalInput")
    sinq = nc.dram_tensor("sinq", [NE, T], DT_B, kind="ExternalInput")
    cosk = nc.dram_tensor("cosk", [NE, KVMAX], DT_B, kind="ExternalInput")
    sink = nc.dram_tensor("sink", [NE, KVMAX], DT_B, kind="ExternalInput")
    partial = nc.dram_tensor("partial", [T, QD], DT_B, kind="ExternalOutput")

    with tile.TileContext(nc) as tc:
        if reps == 1:
            _body(nc, tc, xT, yT, wqT, wkT, wvT, woT, cosq, sinq, cosk,
                  sink, partial)
        else:
            with tc.For_i(0, reps, 1):
                _body(nc, tc, xT, yT, wqT, wkT, wvT, woT, cosq, sinq,
                      cosk, sink, partial)

    nc.compile()
    _compiled[reps] = nc
    return nc


def _rope(nc, pool, dst, cos_sb, sin_sb, w):
    """In-place RoPE on dst[0:NE, :w] (head-dim on partitions).

    cos_sb/sin_sb are [NE, w] slices; sin rows 0:32 carry -sin, 32:64 +sin.
    """
    rot = pool.tile([NE, w], DT_B, tag="rot")
    half = NE // 2
    nc.vector.stream_shuffle(rot[0:half, :], dst[half:NE, :], IDENT32)
    nc.vector.stream_shuffle(rot[half:NE, :], dst[0:half, :], IDENT32)
    nc.vector.tensor_mul(rot[:, :], rot[:, :], sin_sb)
    nc.vector.tensor_mul(dst[0:NE, :], dst[0:NE, :], cos_sb)
    nc.vector.tensor_add(dst[0:NE, :], dst[0:NE, :], rot[:, :])


def _body(nc, tc, xT, yT, wqT, wkT, wvT, woT, cosq, sinq, cosk, sink,
          partial):
    from contextlib import ExitStack

    with ExitStack() as ctx:
        const = ctx.enter_context(tc.tile_pool(name="const", bufs=1))
        qpool = ctx.enter_context(tc.tile_pool(name="q", bufs=1))
        kvpool = ctx.enter_context(tc.tile_pool(name="kv", bufs=1))
        ktab = ctx.enter_context(tc.tile_pool(name="ktab", bufs=1))
        rope_pool = ctx.enter_context(tc.tile_pool(name="rope", bufs=2))
        # PSUM: pps 2 + sps 1x4 + avps 2 = 8 banks
        pps = ctx.enter_context(tc.tile_pool(name="pps", bufs=2,
                                             space="PSUM"))
        sps = ctx.enter_context(tc.tile_pool(name="sps", bufs=1,
                                             space="PSUM"))
        avps = ctx.enter_context(tc.tile_pool(name="avps", bufs=2,
                                              space="PSUM"))

        zbias = const.tile([128, 1], DT_F)
        nc.gpsimd.memset(zbias[:], 0.0)
        ones_col = const.tile([128, 1], DT_B)
        nc.gpsimd.memset(ones_col[:], 1.0)

        q_all = qpool.tile([128, HPC, T], DT_B)
        k_sb = kvpool.tile([128, HPC, KVMAX], DT_B)
        v_sb = kvpool.tile([128, ST, HD], DT_B)
        cosk_sb = ktab.tile([NE, KVMAX], DT_B)
        sink_sb = ktab.tile([NE, KVMAX], DT_B)
        nc.sync.dma_start(cosk_sb[:], cosk.ap())
        nc.sync.dma_start(sink_sb[:], sink.ap())

        # ---- P0: all Q projections (+ RoPE) while x streams ----
        with ExitStack() as c0:
            wqpool = c0.enter_context(tc.tile_pool(name="wq", bufs=1))
            xpool = c0.enter_context(tc.tile_pool(name="x", bufs=2))
            qtab = c0.enter_context(tc.tile_pool(name="qtab", bufs=1))

            wq_sb = wqpool.tile([128, KT, HD], DT_B)
            x_tiles = []
            for _ in range(NCH):
                x_sb = xpool.tile([128, KT, CW], DT_B, tag="x")
                x_tiles.append(x_sb)
            # interleave wq/x0 pieces k-tile-major: the first qproj matmuls
            # (kk=0..3) can start once the first two pieces land
            for g in range(4):
                nc.sync.dma_start(wq_sb[:, g * 4:(g + 1) * 4, :],
                                  wqT.ap()[:, g * 4:(g + 1) * 4, :])
                nc.sync.dma_start(x_tiles[0][:, g * 4:(g + 1) * 4, :],
                                  xT.ap()[:, 0, g * 4:(g + 1) * 4, :])
            cosq_sb = qtab.tile([NE, T], DT_B)
            nc.sync.dma_start(cosq_sb[:], cosq.ap())
            sinq_sb = qtab.tile([NE, T], DT_B)
            nc.sync.dma_start(sinq_sb[:], sinq.ap())
            for c in range(1, NCH):
                for g in range(4):
                    nc.sync.dma_start(x_tiles[c][:, g * 4:(g + 1) * 4, :],
                                      xT.ap()[:, c, g * 4:(g + 1) * 4, :])

            for ci in range(NCH):
                x_sb = x_tiles[ci]
                for h in range(HPC):
                    qp = pps.tile([128, CW], DT_F, tag="pp")
                    for kk in range(KT):
                        nc.tensor.matmul(
                            qp[:], wq_sb[:, kk, h * 128:(h + 1) * 128],
                            x_sb[:, kk, :],
                            start=(kk == 0), stop=(kk == KT - 1))
                    dst = q_all[:, h, ci * CW:(ci + 1) * CW]
                    if h % 2 == 0:
                        nc.vector.tensor_copy(dst, qp[:])
                    else:
                        nc.scalar.copy(dst, qp[:])
                    _rope(nc, rope_pool, dst,
                          cosq_sb[:, ci * CW:(ci + 1) * CW],
                          sinq_sb[:, ci * CW:(ci + 1) * CW], CW)

        # y / wk / wv / wo loads issued behind the P0 x DMAs
        ypool = ctx.enter_context(tc.tile_pool(name="y", bufs=1))
        wkpool = ctx.enter_context(tc.tile_pool(name="wk", bufs=1))
        y_sb = ypool.tile([128, KT, KVMAX], DT_B)
        for g in range(8):
            nc.sync.dma_start(y_sb[:, g * 2:(g + 1) * 2, :],
                              yT.ap()[:, g * 2:(g + 1) * 2, :])
        wk_sb = wkpool.tile([128, KT, HD], DT_B)
        for g in range(4):
            nc.sync.dma_start(wk_sb[:, g * 4:(g + 1) * 4, :],
                              wkT.ap()[:, g * 4:(g + 1) * 4, :])

        with ExitStack() as c1:
            wvpool = c1.enter_context(tc.tile_pool(name="wv", bufs=1))
            wv_sb = wvpool.tile([128, KT, HD], DT_B)
            for g in range(4):
                nc.sync.dma_start(wv_sb[:, g * 4:(g + 1) * 4, :],
                                  wvT.ap()[:, g * 4:(g + 1) * 4, :])

            # ---- P1: V projection; drains on ACT (idle here) ----
            for st in range(ST):
                vp = pps.tile([128, HD], DT_F, tag="pp")
                for kk in range(KT):
                    nc.tensor.matmul(
                        vp[:], y_sb[:, kk, st * 128:(st + 1) * 128],
                        wv_sb[:, kk, :],
                        start=(kk == 0), stop=(kk == KT - 1))
                nc.scalar.copy(v_sb[:, st, :], vp[:])

        # ---- P2: per-head attention with next head's K proj woven in ----
        o_tiles = {}
        opool = ctx.enter_context(tc.tile_pool(name="o", bufs=16))
        wopool = ctx.enter_context(tc.tile_pool(name="wo", bufs=1))
        wo_sb = wopool.tile([128, HPC, QD], DT_B)
        for g in range(HPC):
            nc.sync.dma_start(wo_sb[:, g, :], woT.ap()[:, g, :])
        with ExitStack() as c2:
            apool = c2.enter_context(tc.tile_pool(name="attn", bufs=2))
            lpool = c2.enter_context(tc.tile_pool(name="leaf", bufs=4))
            dpool = c2.enter_context(tc.tile_pool(name="den", bufs=1))
            dsbp = c2.enter_context(tc.tile_pool(name="dsb", bufs=2))

            def kproj_group(h, sc, kk_lo, kk_hi, kp):
                """Emit K-proj matmuls kk_lo..kk_hi for (head h, chunk sc)."""
                for kk in range(kk_lo, kk_hi):
                    nc.tensor.matmul(
                        kp[:], wk_sb[:, kk, h * 128:(h + 1) * 128],
                        y_sb[:, kk, sc * CW:(sc + 1) * CW],
                        start=(kk == 0), stop=(kk == KT - 1))

            def kproj_finish(h, sc, kp):
                dst = k_sb[:, h, sc * CW:(sc + 1) * CW]
                nc.vector.tensor_copy(dst, kp[:])
                _rope(nc, rope_pool, dst,
                      cosk_sb[:, sc * CW:(sc + 1) * CW],
                      sink_sb[:, sc * CW:(sc + 1) * CW], CW)

            # K proj head 0 upfront (drains on DVE; ACT idle)
            for sc in range(NCH):
                kp = pps.tile([128, CW], DT_F, tag="pp")
                kproj_group(0, sc, 0, KT, kp)
                kproj_finish(0, sc, kp)

            # normalize for (c, h) emitted one attention pass later: the
            # GPSIMD tree result p1 isn't ready when its own pass ends, so
            # the denominator matmul would stall PE if issued inline
            pending = []

            def emit_normalize():
                c, h, den_sb, av = pending.pop(0)
                den_bc = dpool.tile([128, CW], DT_F, tag="dbc")
                nc.gpsimd.partition_broadcast(den_bc[:], den_sb[:])
                nc.vector.reciprocal_approx_fast(den_bc[:], den_bc[:])
                o_sb = opool.tile([128, CW], DT_B, tag="o")
                nc.vector.tensor_mul(o_sb[:], den_bc[:], av[:])
                o_tiles[(c, h)] = o_sb

            for h in range(HPC):
                for c in range(NCH):
                    # interleaved K proj (h+1, chunk c): 16 MMs woven into
                    # this attention pass, 2 per exp-paced gap
                    ikp = None
                    if h + 1 < HPC:
                        ikp = pps.tile([128, CW], DT_F, tag="pp")

                    q_ap = q_all[:, h, c * CW:(c + 1) * CW]
                    av = avps.tile([128, CW], DT_F, tag="av")
                    den = pps.tile([1, CW], DT_F, tag="pp")
                    leaves = []
                    at_prev = None
                    for sg in range(ST // 4):
                        sp = sps.tile([128, 4, CW], DT_F, tag="sp")
                        for j in range(4):
                            st = 4 * sg + j
                            nc.tensor.matmul(
                                sp[:, j, :],
                                k_sb[:, h, st * 128:(st + 1) * 128],
                                q_ap, start=True, stop=True)
                        if ikp is not None:
                            kproj_group(h + 1, c, 4 * sg, 4 * sg + 4, ikp)
                        if at_prev is not None:
                            for j in range(4):
                                st = 4 * (sg - 1) + j
                                nc.tensor.matmul(
                                    av[:],
                                    v_sb[:, st, h * 128:(h + 1) * 128],
                                    at_prev[:, j, :], start=(st == 0),
                                    stop=False)
                        # den matmuls lag their leaf by one super-group
                        if len(leaves) >= 2:
                            lp = leaves[len(leaves) - 2]
                            for j in range(2):
                                nc.tensor.matmul(
                                    den[:], ones_col[:], lp[:, j, :],
                                    start=(len(leaves) == 2 and j == 0),
                                    stop=False)
                        if sg == 2 and pending:
                            emit_normalize()
                        at = apool.tile([128, 4, CW], DT_B, tag="at")
                        nc.scalar.activation(
                            at[:, :, :], sp[:, :, :],
                            mybir.ActivationFunctionType.Exp, bias=zbias[:])
                        ps = lpool.tile([128, 2, CW], DT_B, tag="ps")
                        nc.vector.tensor_add(ps[:, :, :], at[:, 0:2, :],
                                             at[:, 2:4, :])
                        leaves.append(ps)
                        at_prev = at
                    # tail: av for the last super-group, then the lagged den
                    # MMs — all before kproj_finish's DVE burst
                    for j in range(4):
                        st = ST - 4 + j
                        nc.tensor.matmul(
                            av[:], v_sb[:, st, h * 128:(h + 1) * 128],
                            at_prev[:, j, :], start=False,
                            stop=(st == ST - 1))
                    for i in (2, 3):
                        for j in range(2):
                            nc.tensor.matmul(
                                den[:], ones_col[:], leaves[i][:, j, :],
                                start=False, stop=(i == 3 and j == 1))
                    den_sb = dsbp.tile([1, CW], DT_F, tag="dsb")
                    nc.scalar.copy(den_sb[:], den[:])
                    if ikp is not None:
                        kproj_finish(h + 1, c, ikp)
                    pending.append((c, h, den_sb, av))

            while pending:
                emit_normalize()

        # ---- P3: output projection; fp tiles rotate over pps+avps
        # (4 single-bank buffers) so drains never gate the matmul stream ----
        with tc.tile_pool(name="part", bufs=3) as ppart:
            for c in range(NCH):
                for qt in range(CW // 128):
                    part_sb = ppart.tile([128, QD], DT_B, tag="part")
                    for nt in range(QD // 512):
                        pool = pps if nt % 2 == 0 else avps
                        tag = "pp" if nt % 2 == 0 else "av"
                        fp = pool.tile([128, 512], DT_F, tag=tag)
                        for h in range(HPC):
                            nc.tensor.matmul(
                                fp[:],
                                o_tiles[(c, h)][:, qt * 128:(qt + 1) * 128],
                                wo_sb[:, h, nt * 512:(nt + 1) * 512],
                                start=(h == 0), stop=(h == HPC - 1))
                        dst = part_sb[:, nt * 512:(nt + 1) * 512]
                        if nt % 2 == 0:
                            nc.vector.tensor_copy(dst, fp[:])
                        else:
                            nc.scalar.copy(dst, fp[:])
                    row0 = c * CW + qt * 128
                    nc.sync.dma_start(partial[row0:row0 + 128, :],
                                      part_sb[:])


def _tile_rows(a, p=128):
    """[R, M] with R = n*p  ->  [p, n, M] (partition-major tiling)."""
    r, m = a.shape
    return np.ascontiguousarray(
        a.reshape(r // p, p, m).transpose(1, 0, 2))


def _host_shards(inputs):
    """Build the 8 per-core input maps from the full inputs."""
    x = np.asarray(inputs["x"], np.float32)
    y = np.asarray(inputs["y"], np.float32)
    rope_cos = np.asarray(inputs["rope_cos"], np.float32)
    rope_sin = np.asarray(inputs["rope_sin"], np.float32)
    wq = np.asarray(inputs["wq"], np.float32)
    wk = np.asarray(inputs["wk"], np.float32)
    wv = np.asarray(inputs["wv"], np.float32)
    wo = np.asarray(inputs["wo"], np.float32)
    input_pos = np.asarray(inputs["input_pos"], np.int64)

    # KV-cache scatter folded into a host-side permutation of y's rows and
    # of the rope tables (k positions live at cache slot input_pos[s]).
    y_cache = np.zeros((B, KVMAX, CD), np.float32)
    y_cache[:, input_pos, :] = y
    ck = np.zeros((KVMAX, NE // 2), np.float32)
    ck[input_pos] = rope_cos
    sk = np.zeros((KVMAX, NE // 2), np.float32)
    sk[input_pos] = rope_sin

    def tabT(cos2, sin2):
        cosT = np.tile(cos2.T, (2, 1)).astype(BF16)          # [NE, S]
        sinT = np.concatenate([-sin2.T, sin2.T], 0).astype(BF16)
        return np.ascontiguousarray(cosT), np.ascontiguousarray(sinT)

    cosq_h, sinq_h = tabT(rope_cos[:T], rope_sin[:T])
    cosk_h, sink_h = tabT(ck, sk)

    in_maps = []
    for core in range(N_CORES):
        b, hg = core // N_HG, core % N_HG
        rows = slice(hg * HD, (hg + 1) * HD)
        xt = _tile_rows(x[b].T.astype(BF16))        # [128, KT, T]
        xt = np.ascontiguousarray(
            xt.reshape(128, KT, NCH, CW).transpose(0, 2, 1, 3))
        in_maps.append({
            "xT": xt,                               # [128, NCH, KT, CW]
            "yT": _tile_rows(y_cache[b].T.astype(BF16)),
            "wqT": _tile_rows((wq[rows] * SCALE).T.astype(BF16)),
            "wkT": _tile_rows(wk[rows].T.astype(BF16)),
            "wvT": _tile_rows(wv[rows].T.astype(BF16)),
            "woT": _tile_rows(wo[:, rows].T.astype(BF16)),
            "cosq": cosq_h, "sinq": sinq_h,
            "cosk": cosk_h, "sink": sink_h,
        })
    return in_maps


def _run(inputs, trace=False, reps=1, **kw):
    nc = _build(reps)
    in_maps = _host_shards(inputs)
    res = run_bass_kernel_spmd(nc, in_maps, list(range(N_CORES)),
                               trace=trace, **kw)
    out = np.zeros((B, T, QD), np.float32)
    for core in range(N_CORES):
        out[core // N_HG] += np.asarray(res.results[core]["partial"],
                                        dtype=np.float32)
    return out, res


def kernel(**inputs):
    out, _ = _run(inputs)
    return out
